# revision 19
# baseline (speedup 1.0000x reference)
"""Causal self-attention (query-axis softmax) for Trainium2, 8 NeuronCores.

Sharding: 8 cores = 4 batches x 2 half-head-groups. Core c handles batch
c//2 and heads (c%2)*6 .. (c%2)*6+5. Each core computes its heads' full
attention plus its partial output projection; the host sums the two
partials per batch and adds b_proj.

Layout strategy per core (T=2048, C=768, 6 heads, hd=64):
  - host passes x[b].T (bf16) so the QKV contraction dim (C) lands on
    SBUF partitions without any on-chip transpose; x and all weights
    travel as bf16 to halve input DMA.
  - Q,K are produced transposed ([head_d, t]) so S^T = K Q^T tiles have
    softmax's query axis on the free dimension; V is produced in [t, d].
  - head_dim=64 tiled matmul modes run at the cold 1.2 GHz clock, so
    the 64-wide operands are zero-padded to full 128x128 mode: K^T
    tiles carry zeros on the other head's partition rows, Vs tiles
    carry 64 zero columns.
  - ScalarE's exp stream is the kernel's critical resource (~170us of
    ACT work vs ~150us of PE work), so attention is split into two
    passes to start it as early as possible: PASS 1 computes S^T+exp
    for the q<1024 half of k-tiles kt<8 (only needs Q[0:1024]/K of one
    pair), with its exp results parked in persistent at_A tiles and
    partial row-sums in persistent accumulators; it starts ~10us in,
    right after the first pair's Q/K, and is interleaved
    instruction-by-instruction with the remaining QKV matmuls (the
    "filler" queue) so the in-order PE never stalls on the exp chain.
    PASS 2 computes the q>=1024 logits, completes the denominators,
    and emits the normalization + AV + projection. AV for k-tile kt is
    emitted a few tiles behind S^T (pend depth 4, carried across pair
    boundaries so the PE pipeline never drains).
  - softmax over q (free axis): no max-subtraction needed (logits are
    O(1) by construction); exp+rowsum fused on ScalarE via accum_out;
    normalization folded into V rows (scale V[k,:] by 1/denom[k]).
  - causal mask: ragged chunk bounds skip fully-masked blocks; diagonal
    128x128 blocks get a triangular -30000 add applied ON THE PE (an
    identity-stationary matmul accumulating the precomputed triangle
    into the S^T psum group), keeping VectorE out of the S^T->exp
    dependency chain.
  - attention tensors (Q,K,at,V,y,wp) are bf16 (softmax normalization
    cancels most rounding; measured ~0.5% output effect, tol is 2e-2);
    psum accumulation stays fp32.
"""

import os
import sys

sys.path.insert(0, "/opt/trn_rl_repo")

import ml_dtypes
import numpy as np

import concourse.bass as bass
import concourse.mybir as mybir
import concourse.tile as tile
from concourse import bass_utils
from concourse.bass_utils import run_bass_kernel_spmd

# walrus --enable-ldw-opt crashes on fp32r ldweights; keep it off (the
# LDWEIGHTS are fully hidden behind matmul streaming anyway).
LDW_OPT = {"on": os.environ.get("ATT_LDWOPT", "0") == "1"}
if not getattr(bass_utils, "_ldw_opt_patched", False):
    _orig_run_command = bass_utils.run_command

    def _run_command_ldw(cmd, *a, **kw):
        if LDW_OPT["on"]:
            cmd = [
                "--enable-ldw-opt=true" if c == "--enable-ldw-opt=false" else c
                for c in cmd
            ]
        return _orig_run_command(cmd, *a, **kw)

    bass_utils.run_command = _run_command_ldw
    bass_utils._ldw_opt_patched = True

FP32 = mybir.dt.float32
FP32R = mybir.dt.float32r
BF16 = mybir.dt.bfloat16
U16 = mybir.dt.uint16

B, T, C, H = 4, 2048, 768, 12
D = 64                  # head dim
NCORES = 8
HPC = H * B // NCORES   # heads per core = 6
E = HPC * D             # qkv slice width per core = 384
CT = C // 128           # c tiles = 6
ET = E // 128           # e tiles = 3
TT = T // 128           # t tiles = 16
QCH = 512               # matmul moving chunk (PSUM bank limit)
NQC = T // QCH          # 4
BCH = 1024              # exp chunk
NBC = T // BCH          # 2
MASKV = -30000.0
SCALE = 1.0 / 8.0       # 1/sqrt(hd)
PEND = 6                # AV emission lag (S^T/exp run-ahead depth)
Exp = mybir.ActivationFunctionType.Exp
AxX = mybir.AxisListType.X


def _split_sync_waits(nc):
    """This container's walrus encodes at most one sync wait per
    instruction for several instruction structs; hoist extra waits onto
    same-engine nops placed immediately before the instruction."""
    for f in nc.m.functions:
        for bb in f.blocks:
            new_insts = []
            for inst in bb.instructions:
                si = inst.sync_info
                waits = list(si.on_wait) if si is not None and si.on_wait else []
                if len(waits) > 1:
                    for w in waits[:-1]:
                        nop = mybir.InstNoOp(
                            name=nc.get_next_instruction_name(),
                            engine=inst.engine,
                            sync_info=mybir.SyncInfo(on_wait=[w], on_update=[]),
                            bass_nofuse=True,
                        )
                        nc.register_instruction(nop)
                        new_insts.append(nop)
                    inst.sync_info = mybir.SyncInfo(
                        on_wait=[waits[-1]], on_update=list(si.on_update or [])
                    )
                new_insts.append(inst)
            bb.instructions[:] = new_insts


def _build():
    nc = bass.Bass("TRN2")
    xT = nc.dram_tensor("xT", [NQC, 128, CT, QCH], BF16, kind="ExternalInput")
    wq = nc.dram_tensor("wq", [128, CT, E], BF16, kind="ExternalInput")
    wk = nc.dram_tensor("wk", [128, CT, E], BF16, kind="ExternalInput")
    wv = nc.dram_tensor("wv", [128, CT, E], BF16, kind="ExternalInput")
    bq = nc.dram_tensor("bq", [E], FP32, kind="ExternalInput")
    bk = nc.dram_tensor("bk", [E], FP32, kind="ExternalInput")
    bv = nc.dram_tensor("bv", [E], FP32, kind="ExternalInput")
    wp = nc.dram_tensor("wp", [128, ET, C], BF16, kind="ExternalInput")
    mask = nc.dram_tensor("mask", [128, 128], BF16, kind="ExternalInput")
    ident = nc.dram_tensor("ident", [128, 128], BF16, kind="ExternalInput")
    out = nc.dram_tensor("out", [T, C], FP32, kind="ExternalOutput")

    with tile.TileContext(nc) as tc:
        with (
            tc.tile_pool(name="wts", bufs=1) as wts,
            tc.tile_pool(name="xp", bufs=1) as xp,
            tc.tile_pool(name="big", bufs=1) as big,
            tc.tile_pool(name="atp", bufs=1) as atp,
            tc.tile_pool(name="sm", bufs=4) as sm,
            tc.tile_pool(name="op", bufs=3) as op,
        ):
            # ---- constant loads ----
            # ordered so pass 1 of pair 0 (xt0/xt1, wq, wk, mask) can
            # start as early as possible
            xt_pre = [xp.tile([128, CT, QCH], BF16, tag="xt", bufs=4, name="xt")
                      for _ in range(NQC)]
            nc.sync.dma_start(out=xt_pre[0], in_=xT[0])
            wq_sb = wts.tile([128, CT, E], BF16)
            wk_sb = wts.tile([128, CT, E], BF16)
            wv_sb = wts.tile([128, CT, E], BF16)
            nc.sync.dma_start(out=wq_sb, in_=wq[:])
            nc.sync.dma_start(out=wk_sb, in_=wk[:])
            mask_sb = wts.tile([128, 128], BF16)
            nc.sync.dma_start(out=mask_sb, in_=mask[:])
            ident_sb = wts.tile([128, 128], BF16)
            nc.sync.dma_start(out=ident_sb, in_=ident[:])
            bq_sb = wts.tile([128, ET], FP32)
            bk_sb = wts.tile([128, ET], FP32)
            nc.sync.dma_start(out=bq_sb, in_=bq.rearrange("(et p) -> p et", p=128))
            nc.sync.dma_start(out=bk_sb, in_=bk.rearrange("(et p) -> p et", p=128))
            nc.sync.dma_start(out=xt_pre[1], in_=xT[1])
            nc.sync.dma_start(out=xt_pre[2], in_=xT[2])
            nc.sync.dma_start(out=xt_pre[3], in_=xT[3])
            nc.sync.dma_start(out=wv_sb, in_=wv[:])
            bv_sb = wts.tile([128, E], FP32)
            nc.sync.dma_start(out=bv_sb, in_=bv[None, :].to_broadcast((128, E)))
            wp_sb = wts.tile([128, ET, C], BF16)
            nc.sync.dma_start(out=wp_sb, in_=wp[:])

            qt2 = big.tile([128, ET, T], BF16)      # [d-in-pair, pair, t]
            ktp2 = big.tile([128, ET, 2, T], BF16)  # [d(+zero half), pair, head-in-pair, t]
            v_sb = big.tile([128, TT, E], BF16)     # [t-in-tile, ttile, (head,d)]
            y_sb = big.tile([128, ET, T], BF16)     # [hd-in-pair, pair, t]
            nc.gpsimd.memset(ktp2.bitcast(U16), 0)
            # persistent rotating Vs tiles, 4 per head-in-pair slot; head A
            # tiles keep cols 64:128 zero, head B tiles keep cols 0:64 zero.
            vspad = [
                [big.tile([128, 128], BF16, name=f"vspad{j}_{i}") for i in range(5)]
                for j in range(2)
            ]
            for row in vspad:
                for t_ in row:
                    nc.gpsimd.memset(t_.bitcast(U16), 0)
            # pass-1 persistent state: exp(q<1024) results and partial sums
            at_A = {}
            sums2 = {}
            for hp in range(ET):
                for kt in range(8):
                    for hj in range(2):
                        at_A[hp, kt, hj] = big.tile(
                            [128, BCH - 128 * kt], BF16, name=f"atA_{hp}_{kt}_{hj}"
                        )
                        sums2[hp, kt, hj] = big.tile(
                            [128, 2], FP32, name=f"sums2_{hp}_{kt}_{hj}"
                        )

            # ---- QKV units (emitted as fillers between pass-1 steps) ----
            def emit_q(psA, et, tci):
                xt = xt_pre[tci]
                cols = slice(tci * QCH, (tci + 1) * QCH)
                pq = psA.tile([128, QCH], FP32, tag="ps", bufs=4, name="pq")
                for ct in range(CT):
                    nc.tensor.matmul(
                        pq, wq_sb[:, ct, et * 128:(et + 1) * 128], xt[:, ct, :],
                        start=(ct == 0), stop=(ct == CT - 1),
                    )
                nc.vector.tensor_scalar_add(qt2[:, et, cols], pq, bq_sb[:, et:et + 1])

            def emit_k(psA, et, tci):
                xt = xt_pre[tci]
                cols = slice(tci * QCH, (tci + 1) * QCH)
                pk = psA.tile([128, QCH], FP32, tag="ps", bufs=4, name="pk")
                for ct in range(CT):
                    nc.tensor.matmul(
                        pk, wk_sb[:, ct, et * 128:(et + 1) * 128], xt[:, ct, :],
                        start=(ct == 0), stop=(ct == CT - 1),
                    )
                nc.vector.tensor_scalar_add(
                    ktp2[0:64, et, 0, cols], pk[0:64, :], bk_sb[0:64, et:et + 1]
                )
                nc.vector.tensor_scalar_add(
                    ktp2[64:128, et, 1, cols], pk[64:128, :], bk_sb[64:128, et:et + 1]
                )

            def emit_v(psA, tci, ttl):
                xt = xt_pre[tci]
                tt = tci * 4 + ttl
                pv = psA.tile([128, QCH], FP32, tag="ps", bufs=4, name="pv")
                for ct in range(CT):
                    nc.tensor.matmul(
                        pv[:, :E], xt[:, ct, ttl * 128:(ttl + 1) * 128], wv_sb[:, ct, :],
                        start=(ct == 0), stop=(ct == CT - 1),
                    )
                nc.vector.tensor_add(v_sb[:, tt, :], pv[:, :E], bv_sb)

            def st_chunk(psS, hp, hj, kt, bc, with_diag):
                """S^T matmuls for chunk bc of k-tile kt (+mask on diag);
                returns the psum tile."""
                klo = 128 * kt
                blo = max(BCH * bc, klo)
                s_ps = psS.tile([128, BCH], FP32, tag="s", bufs=2, name="s_ps")
                for half in range(2):
                    plo = max(blo, BCH * bc + half * QCH)
                    phi = BCH * bc + (half + 1) * QCH
                    if plo >= phi:
                        continue
                    nc.tensor.matmul(
                        s_ps[:, plo - BCH * bc:phi - BCH * bc],
                        ktp2[:, hp, hj, klo:klo + 128],
                        qt2[:, hp, plo:phi],
                        start=True,
                        stop=not (with_diag and half == klo // QCH % 2),
                        skip_group_check=True,
                    )
                if with_diag:
                    off = klo - BCH * bc
                    nc.tensor.matmul(
                        s_ps[:, off:off + 128], ident_sb, mask_sb,
                        start=False, stop=True, skip_group_check=True,
                    )
                return s_ps

            # ---- phase A: QKV + attention pass 1 (q<1024, kt<8) ----
            with tc.tile_pool(name="psS", bufs=2, space="PSUM") as psS:
                with tc.tile_pool(name="psA", bufs=4, space="PSUM") as psA:
                    emit_q(psA, 0, 0)
                    emit_k(psA, 0, 0)
                    emit_q(psA, 0, 1)
                    fillers = []
                    fillers += [
                        lambda p=psA: emit_q(p, 0, 2),
                        lambda p=psA: emit_k(p, 0, 1),
                        lambda p=psA: emit_q(p, 0, 3),
                        lambda p=psA: emit_k(p, 0, 2),
                        lambda p=psA: emit_k(p, 0, 3),
                    ]
                    ready_mark = {}  # hp -> filler index that must be drained
                    for et in range(1, ET):
                        for tci in range(NQC):
                            fillers.append(lambda p=psA, e=et, t=tci: emit_q(p, e, t))
                            fillers.append(lambda p=psA, e=et, t=tci: emit_k(p, e, t))
                            if tci == 1:
                                ready_mark[et] = len(fillers)
                    for tci in range(NQC):
                        for ttl in range(4):
                            fillers.append(lambda p=psA, t=tci, l=ttl: emit_v(p, t, l))

                    nfill = len(fillers)
                    filled = [0]

                    def fill_to(n):
                        while filled[0] < min(n, nfill):
                            fillers[filled[0]]()
                            filled[0] += 1

                    giter = [0]
                    NITER = ET * 8 * 2  # pass-1 iterations total

                    for hp in range(ET):
                        if hp > 0:
                            fill_to(ready_mark[hp])
                        for kt in range(8):
                            klo = 128 * kt
                            for hj in range(2):
                                s_ps = st_chunk(psS, hp, hj, kt, 0, True)
                                nc.scalar.activation(
                                    at_A[hp, kt, hj][:, 0:BCH - klo],
                                    s_ps[:, klo:],
                                    Exp, scale=SCALE,
                                    accum_out=sums2[hp, kt, hj][:, 0:1],
                                )
                                giter[0] += 1
                                fill_to((giter[0] * nfill) // NITER)
                    fill_to(nfill)

                # ---- phase B: attention pass 2 (q>=1024) + AV ----
                # The two heads of a pair interleave their kt loops and
                # SHARE one y^T psum tile: head A's padded AV writes zeros
                # to rows 64:128 (B's rows) and vice versa, so rows 0:64 =
                # head A's y^T and 64:128 = head B's. pend carries across
                # pairs so the PE pipeline never drains at a pair boundary.
                with tc.tile_pool(name="psY", bufs=1, space="PSUM") as psY:
                    pend = []  # [(hp, yps, hj, kt, atA, atB, vsp)]
                    n_av = [0] * ET

                    def emit_av(hp0, yps0, hj, kt0, atA0, atB0, vsp0):
                        klo0 = 128 * kt0
                        for qc in range(kt0 // 4, NQC):
                            lo = max(QCH * qc, klo0)
                            hi = QCH * qc + QCH
                            if qc < 2:
                                mov = atA0[:, lo - klo0:hi - klo0]
                            else:
                                mov = atB0[:, lo - BCH:hi - BCH]
                            nc.tensor.matmul(
                                yps0[:, lo:hi], vsp0, mov,
                                start=(kt0 == 0 and hj == 0),
                                stop=(kt0 == min(TT - 1, 4 * qc + 3) and hj == 1),
                                skip_group_check=True,
                            )
                        n_av[hp0] += 1
                        if n_av[hp0] == 16:
                            # all kt<8 AVs emitted: yps[:, :1024] is final
                            nc.vector.tensor_copy(y_sb[:, hp0, 0:BCH], yps0[:, 0:BCH])
                        elif n_av[hp0] == 2 * TT:
                            nc.vector.tensor_copy(y_sb[:, hp0, BCH:T], yps0[:, BCH:T])

                    for hp in range(ET):
                        yps = psY.tile([128, T], FP32, tag="y", name="yps")
                        for kt in range(TT):
                            klo = 128 * kt
                            for hj in range(2):
                                hl = 2 * hp + hj
                                atB = atp.tile([128, BCH], BF16, tag="atB", bufs=8, name="atB")
                                rcp = sm.tile([128, 1], FP32, tag="rcp", bufs=6, name="rcp")
                                stot = sm.tile([128, 1], FP32, tag="stot", bufs=6, name="stot")
                                # row-sums on VectorE (from the bf16 at tile)
                                # keep the saturated ScalarE free of
                                # ACTIVATION_READ_ACCUMULATOR overhead
                                if kt < 8:
                                    s_ps = st_chunk(psS, hp, hj, kt, 1, False)
                                    nc.scalar.activation(atB, s_ps, Exp, scale=SCALE)
                                    nc.vector.reduce_sum(stot, atB, axis=AxX)
                                    stot2 = sm.tile([128, 1], FP32, tag="stot2", bufs=6, name="stot2")
                                    nc.vector.tensor_add(
                                        stot2, stot, sums2[hp, kt, hj][:, 0:1]
                                    )
                                    nc.vector.reciprocal(rcp, stot2)
                                else:
                                    s_ps = st_chunk(psS, hp, hj, kt, 1, True)
                                    nc.scalar.activation(
                                        atB[:, klo - BCH:], s_ps[:, klo - BCH:],
                                        Exp, scale=SCALE,
                                    )
                                    nc.vector.reduce_sum(
                                        stot, atB[:, klo - BCH:], axis=AxX
                                    )
                                    nc.vector.reciprocal(rcp, stot)
                                # slot index must be continuous across the
                                # pair boundary: AV(pair p, kt15) is emitted
                                # during pair p+1's early iterations, after
                                # p+1's first vsp writes
                                vsp = vspad[hj][(hp * TT + kt) % 5]
                                nc.vector.tensor_scalar_mul(
                                    vsp[:, hj * 64:hj * 64 + 64],
                                    v_sb[:, kt, hl * 64:(hl + 1) * 64], rcp
                                )
                                pend.append(
                                    (hp, yps, hj, kt, at_A.get((hp, kt, hj)), atB, vsp)
                                )
                                if len(pend) > PEND:
                                    emit_av(*pend.pop(0))
                    for p_ in pend:
                        emit_av(*p_)

            # ---- output projection ----
            with tc.tile_pool(name="psP", bufs=4, space="PSUM") as psP:
                for tt in range(TT):
                    po1 = psP.tile([128, QCH], FP32, tag="ps", bufs=4, name="po1")
                    po2 = psP.tile([128, QCH], FP32, tag="ps", bufs=4, name="po2")
                    for et in range(ET):
                        nc.tensor.matmul(
                            po1, y_sb[:, et, tt * 128:(tt + 1) * 128], wp_sb[:, et, 0:QCH],
                            start=(et == 0), stop=(et == ET - 1),
                        )
                        nc.tensor.matmul(
                            po2[:, :C - QCH], y_sb[:, et, tt * 128:(tt + 1) * 128],
                            wp_sb[:, et, QCH:C],
                            start=(et == 0), stop=(et == ET - 1),
                        )
                    o_sb = op.tile([128, C], FP32, tag="o", bufs=3, name="o_sb")
                    nc.vector.tensor_copy(o_sb[:, 0:QCH], po1)
                    nc.scalar.copy(o_sb[:, QCH:C], po2[:, :C - QCH])
                    nc.sync.dma_start(out=out[tt * 128:(tt + 1) * 128, :], in_=o_sb)

    _split_sync_waits(nc)
    return nc


_nc_cache = {}
last_result = None


def kernel(x, w_attn, b_attn, w_proj, b_proj):
    global last_result
    if "nc" not in _nc_cache:
        _nc_cache["nc"] = _build()
    nc = _nc_cache["nc"]

    x = np.asarray(x, dtype=np.float32)
    w_attn = np.asarray(w_attn, dtype=np.float32)
    b_attn = np.asarray(b_attn, dtype=np.float32)
    w_proj = np.asarray(w_proj, dtype=np.float32)
    b_proj = np.asarray(b_proj, dtype=np.float32)

    bf16 = ml_dtypes.bfloat16
    tri = np.where(
        np.arange(128)[None, :] >= np.arange(128)[:, None], 0.0, MASKV
    ).astype(bf16)
    eye = np.eye(128, dtype=np.float32).astype(bf16)

    in_maps = []
    for core in range(NCORES):
        b = core // 2
        e0 = (core % 2) * E
        xt_host = np.ascontiguousarray(
            x[b].T.reshape(CT, 128, NQC, QCH).transpose(2, 1, 0, 3)
        ).astype(bf16)
        def _wblk(w):
            return np.ascontiguousarray(
                w.reshape(CT, 128, E).transpose(1, 0, 2)
            ).astype(bf16)
        in_maps.append({
            "xT": xt_host,
            "wq": _wblk(w_attn[:, e0:e0 + E]),
            "wk": _wblk(w_attn[:, C + e0:C + e0 + E]),
            "wv": _wblk(w_attn[:, 2 * C + e0:2 * C + e0 + E]),
            "bq": np.ascontiguousarray(b_attn[e0:e0 + E]),
            "bk": np.ascontiguousarray(b_attn[C + e0:C + e0 + E]),
            "bv": np.ascontiguousarray(b_attn[2 * C + e0:2 * C + e0 + E]),
            "wp": np.ascontiguousarray(
                w_proj[e0:e0 + E, :].reshape(ET, 128, C).transpose(1, 0, 2)
            ).astype(bf16),
            "mask": tri,
            "ident": eye,
        })

    trace = os.environ.get("ATT_TRACE", "0")
    kw = {}
    if trace != "0":
        n = min(int(trace), NCORES)
        kw = dict(trace=True, trace_cores=list(range(n)))
    res = run_bass_kernel_spmd(nc, in_maps, list(range(NCORES)), **kw)
    last_result = res

    out = np.zeros((B, T, C), dtype=np.float32)
    for core in range(NCORES):
        out[core // 2] += res.results[core]["out"]
    out += b_proj[None, None, :]
    return out


# revision 20
# speedup vs baseline: 1.0981x; 1.0981x over previous
"""Causal self-attention (query-axis softmax) for Trainium2, 8 NeuronCores.

Sharding: 8 cores = 4 batches x 2 half-head-groups. Core c handles batch
c//2 and heads (c%2)*6 .. (c%2)*6+5. Each core computes its heads' full
attention plus its partial output projection; the host sums the two
partials per batch and adds b_proj.

Layout strategy per core (T=2048, C=768, 6 heads, hd=64):
  - host passes x[b].T (bf16) so the QKV contraction dim (C) lands on
    SBUF partitions without any on-chip transpose; x and all weights
    travel as bf16 to halve input DMA.
  - Q,K are produced transposed ([head_d, t]) so S^T = K Q^T tiles have
    softmax's query axis on the free dimension; V is produced in [t, d].
  - head_dim=64 tiled matmul modes run at the cold 1.2 GHz clock, so
    the 64-wide operands are zero-padded to full 128x128 mode: K^T
    tiles carry zeros on the other head's partition rows, Vs tiles
    carry 64 zero columns.
  - ScalarE's exp stream is the kernel's critical resource (~170us of
    ACT work vs ~150us of PE work), so attention is split into two
    passes to start it as early as possible: PASS 1 computes S^T+exp
    for the q<1024 half of k-tiles kt<8 (only needs Q[0:1024]/K of one
    pair), with its exp results parked in persistent at_A tiles and
    partial row-sums in persistent accumulators; it starts ~10us in,
    right after the first pair's Q/K, and is interleaved
    instruction-by-instruction with the remaining QKV matmuls (the
    "filler" queue) so the in-order PE never stalls on the exp chain.
    PASS 2 computes the q>=1024 logits, completes the denominators,
    and emits the normalization + AV + projection. AV for k-tile kt is
    emitted a few tiles behind S^T (pend depth 4, carried across pair
    boundaries so the PE pipeline never drains).
  - softmax over q (free axis): no max-subtraction needed (logits are
    O(1) by construction); exp+rowsum fused on ScalarE via accum_out;
    normalization folded into V rows (scale V[k,:] by 1/denom[k]).
  - causal mask: ragged chunk bounds skip fully-masked blocks; diagonal
    128x128 blocks get a triangular -30000 add applied ON THE PE (an
    identity-stationary matmul accumulating the precomputed triangle
    into the S^T psum group), keeping VectorE out of the S^T->exp
    dependency chain.
  - attention tensors (Q,K,at,V,y,wp) are bf16 (softmax normalization
    cancels most rounding; measured ~0.5% output effect, tol is 2e-2);
    psum accumulation stays fp32.
"""

import os
import sys

sys.path.insert(0, "/opt/trn_rl_repo")

import ml_dtypes
import numpy as np

import concourse.bass as bass
import concourse.mybir as mybir
import concourse.tile as tile
from concourse import bass_utils
from concourse.bass_utils import run_bass_kernel_spmd

# walrus --enable-ldw-opt crashes on fp32r ldweights; keep it off (the
# LDWEIGHTS are fully hidden behind matmul streaming anyway).
LDW_OPT = {"on": os.environ.get("ATT_LDWOPT", "0") == "1"}
if not getattr(bass_utils, "_ldw_opt_patched", False):
    _orig_run_command = bass_utils.run_command

    def _run_command_ldw(cmd, *a, **kw):
        if LDW_OPT["on"]:
            cmd = [
                "--enable-ldw-opt=true" if c == "--enable-ldw-opt=false" else c
                for c in cmd
            ]
        return _orig_run_command(cmd, *a, **kw)

    bass_utils.run_command = _run_command_ldw
    bass_utils._ldw_opt_patched = True

FP32 = mybir.dt.float32
FP32R = mybir.dt.float32r
BF16 = mybir.dt.bfloat16
U16 = mybir.dt.uint16

B, T, C, H = 4, 2048, 768, 12
D = 64                  # head dim
NCORES = 8
HPC = H * B // NCORES   # heads per core = 6
E = HPC * D             # qkv slice width per core = 384
CT = C // 128           # c tiles = 6
ET = E // 128           # e tiles = 3
TT = T // 128           # t tiles = 16
QCH = 512               # matmul moving chunk (PSUM bank limit)
NQC = T // QCH          # 4
BCH = 1024              # exp chunk
NBC = T // BCH          # 2
MASKV = -30000.0
SCALE = 1.0 / 8.0       # 1/sqrt(hd)
PEND = 6                # AV emission lag (S^T/exp run-ahead depth)
Exp = mybir.ActivationFunctionType.Exp
AxX = mybir.AxisListType.X


def _split_sync_waits(nc):
    """This container's walrus encodes at most one sync wait per
    instruction for several instruction structs; hoist extra waits onto
    same-engine nops placed immediately before the instruction."""
    for f in nc.m.functions:
        for bb in f.blocks:
            new_insts = []
            for inst in bb.instructions:
                si = inst.sync_info
                waits = list(si.on_wait) if si is not None and si.on_wait else []
                if len(waits) > 1:
                    for w in waits[:-1]:
                        nop = mybir.InstNoOp(
                            name=nc.get_next_instruction_name(),
                            engine=inst.engine,
                            sync_info=mybir.SyncInfo(on_wait=[w], on_update=[]),
                            bass_nofuse=True,
                        )
                        nc.register_instruction(nop)
                        new_insts.append(nop)
                    inst.sync_info = mybir.SyncInfo(
                        on_wait=[waits[-1]], on_update=list(si.on_update or [])
                    )
                new_insts.append(inst)
            bb.instructions[:] = new_insts


def _build():
    nc = bass.Bass("TRN2")
    xT = nc.dram_tensor("xT", [NQC, 128, CT, QCH], BF16, kind="ExternalInput")
    wq = nc.dram_tensor("wq", [128, CT, E], BF16, kind="ExternalInput")
    wk = nc.dram_tensor("wk", [128, CT, E], BF16, kind="ExternalInput")
    wv = nc.dram_tensor("wv", [128, CT, E], BF16, kind="ExternalInput")
    bq = nc.dram_tensor("bq", [E], FP32, kind="ExternalInput")
    bk = nc.dram_tensor("bk", [E], FP32, kind="ExternalInput")
    bv = nc.dram_tensor("bv", [E], FP32, kind="ExternalInput")
    wp = nc.dram_tensor("wp", [128, ET, C], BF16, kind="ExternalInput")
    mask = nc.dram_tensor("mask", [128, 128], BF16, kind="ExternalInput")
    ident = nc.dram_tensor("ident", [128, 128], BF16, kind="ExternalInput")
    out = nc.dram_tensor("out", [T, C], FP32, kind="ExternalOutput")

    with tile.TileContext(nc) as tc:
        with (
            tc.tile_pool(name="wts", bufs=1) as wts,
            tc.tile_pool(name="xp", bufs=1) as xp,
            tc.tile_pool(name="big", bufs=1) as big,
            tc.tile_pool(name="atp", bufs=1) as atp,
            tc.tile_pool(name="sm", bufs=4) as sm,
            tc.tile_pool(name="op", bufs=3) as op,
        ):
            # ---- constant loads ----
            # ordered so pass 1 of pair 0 (xt0/xt1, wq, wk, mask) can
            # start as early as possible
            xt_pre = [xp.tile([128, CT, QCH], BF16, tag="xt", bufs=4, name="xt")
                      for _ in range(NQC)]
            nc.sync.dma_start(out=xt_pre[0], in_=xT[0])
            wq_sb = wts.tile([128, CT, E], BF16)
            wk_sb = wts.tile([128, CT, E], BF16)
            wv_sb = wts.tile([128, CT, E], BF16)
            nc.sync.dma_start(out=wq_sb, in_=wq[:])
            nc.sync.dma_start(out=wk_sb, in_=wk[:])
            mask_sb = wts.tile([128, 128], BF16)
            nc.sync.dma_start(out=mask_sb, in_=mask[:])
            ident_sb = wts.tile([128, 128], BF16)
            nc.sync.dma_start(out=ident_sb, in_=ident[:])
            bq_sb = wts.tile([128, ET], FP32)
            bk_sb = wts.tile([128, ET], FP32)
            nc.sync.dma_start(out=bq_sb, in_=bq.rearrange("(et p) -> p et", p=128))
            nc.sync.dma_start(out=bk_sb, in_=bk.rearrange("(et p) -> p et", p=128))
            nc.sync.dma_start(out=xt_pre[1], in_=xT[1])
            nc.sync.dma_start(out=xt_pre[2], in_=xT[2])
            nc.sync.dma_start(out=xt_pre[3], in_=xT[3])
            nc.sync.dma_start(out=wv_sb, in_=wv[:])
            bv_sb = wts.tile([128, E], FP32)
            nc.sync.dma_start(out=bv_sb, in_=bv[None, :].to_broadcast((128, E)))
            wp_sb = wts.tile([128, ET, C], BF16)
            nc.sync.dma_start(out=wp_sb, in_=wp[:])

            qt2 = big.tile([128, ET, T], BF16)      # [d-in-pair, pair, t]
            ktp2 = big.tile([128, ET, 2, T], BF16)  # [d(+zero half), pair, head-in-pair, t]
            v_sb = big.tile([128, TT, E], BF16)     # [t-in-tile, ttile, (head,d)]
            y_sb = big.tile([128, ET, T], BF16)     # [hd-in-pair, pair, t]
            nc.gpsimd.memset(ktp2.bitcast(U16), 0)
            # persistent rotating Vs tiles, 4 per head-in-pair slot; head A
            # tiles keep cols 64:128 zero, head B tiles keep cols 0:64 zero.
            vspad = [
                [big.tile([128, 128], BF16, name=f"vspad{j}_{i}") for i in range(5)]
                for j in range(2)
            ]
            for row in vspad:
                for t_ in row:
                    nc.gpsimd.memset(t_.bitcast(U16), 0)
            # pass-1 persistent state: exp(q<1024) results and partial sums
            at_A = {}
            sums2 = {}
            for hp in range(ET):
                for kt in range(8):
                    for hj in range(2):
                        at_A[hp, kt, hj] = big.tile(
                            [128, BCH - 128 * kt], BF16, name=f"atA_{hp}_{kt}_{hj}"
                        )
                        sums2[hp, kt, hj] = big.tile(
                            [128, 2], FP32, name=f"sums2_{hp}_{kt}_{hj}"
                        )

            # ---- QKV units (emitted as fillers between pass-1 steps) ----
            def emit_q(psA, et, tci):
                xt = xt_pre[tci]
                cols = slice(tci * QCH, (tci + 1) * QCH)
                pq = psA.tile([128, QCH], FP32, tag="ps", bufs=4, name="pq")
                for ct in range(CT):
                    nc.tensor.matmul(
                        pq, wq_sb[:, ct, et * 128:(et + 1) * 128], xt[:, ct, :],
                        start=(ct == 0), stop=(ct == CT - 1),
                    )
                nc.vector.tensor_scalar_add(qt2[:, et, cols], pq, bq_sb[:, et:et + 1])

            def emit_k(psA, et, tci):
                xt = xt_pre[tci]
                cols = slice(tci * QCH, (tci + 1) * QCH)
                pk = psA.tile([128, QCH], FP32, tag="ps", bufs=4, name="pk")
                for ct in range(CT):
                    nc.tensor.matmul(
                        pk, wk_sb[:, ct, et * 128:(et + 1) * 128], xt[:, ct, :],
                        start=(ct == 0), stop=(ct == CT - 1),
                    )
                nc.vector.tensor_scalar_add(
                    ktp2[0:64, et, 0, cols], pk[0:64, :], bk_sb[0:64, et:et + 1]
                )
                nc.vector.tensor_scalar_add(
                    ktp2[64:128, et, 1, cols], pk[64:128, :], bk_sb[64:128, et:et + 1]
                )

            def emit_v(psA, tci, ttl):
                xt = xt_pre[tci]
                tt = tci * 4 + ttl
                pv = psA.tile([128, QCH], FP32, tag="ps", bufs=4, name="pv")
                for ct in range(CT):
                    nc.tensor.matmul(
                        pv[:, :E], xt[:, ct, ttl * 128:(ttl + 1) * 128], wv_sb[:, ct, :],
                        start=(ct == 0), stop=(ct == CT - 1),
                    )
                nc.vector.tensor_add(v_sb[:, tt, :], pv[:, :E], bv_sb)

            def st_chunk(psS, hp, hj, kt, bc, with_diag):
                """S^T matmuls for chunk bc of k-tile kt (+mask on diag);
                returns the psum tile."""
                klo = 128 * kt
                blo = max(BCH * bc, klo)
                s_ps = psS.tile([128, BCH], FP32, tag="s", bufs=2, name="s_ps")
                for half in range(2):
                    plo = max(blo, BCH * bc + half * QCH)
                    phi = BCH * bc + (half + 1) * QCH
                    if plo >= phi:
                        continue
                    nc.tensor.matmul(
                        s_ps[:, plo - BCH * bc:phi - BCH * bc],
                        ktp2[:, hp, hj, klo:klo + 128],
                        qt2[:, hp, plo:phi],
                        start=True,
                        stop=not (with_diag and half == klo // QCH % 2),
                        skip_group_check=True,
                    )
                if with_diag:
                    off = klo - BCH * bc
                    nc.tensor.matmul(
                        s_ps[:, off:off + 128], ident_sb, mask_sb,
                        start=False, stop=True, skip_group_check=True,
                    )
                return s_ps

            # ---- phase A: QKV + attention pass 1 (q<1024, kt<8) ----
            with tc.tile_pool(name="psS", bufs=2, space="PSUM") as psS:
                with tc.tile_pool(name="psA", bufs=4, space="PSUM") as psA:
                    emit_q(psA, 0, 0)
                    emit_k(psA, 0, 0)
                    emit_q(psA, 0, 1)
                    fillers = []
                    fillers += [
                        lambda p=psA: emit_q(p, 0, 2),
                        lambda p=psA: emit_k(p, 0, 1),
                        lambda p=psA: emit_q(p, 0, 3),
                        lambda p=psA: emit_k(p, 0, 2),
                        lambda p=psA: emit_k(p, 0, 3),
                    ]
                    ready_mark = {}  # hp -> filler index that must be drained
                    for et in range(1, ET):
                        for tci in range(NQC):
                            fillers.append(lambda p=psA, e=et, t=tci: emit_q(p, e, t))
                            fillers.append(lambda p=psA, e=et, t=tci: emit_k(p, e, t))
                            if tci == 1:
                                ready_mark[et] = len(fillers)
                    for tci in range(NQC):
                        for ttl in range(4):
                            fillers.append(lambda p=psA, t=tci, l=ttl: emit_v(p, t, l))

                    nfill = len(fillers)
                    filled = [0]

                    def fill_to(n):
                        while filled[0] < min(n, nfill):
                            fillers[filled[0]]()
                            filled[0] += 1

                    giter = [0]
                    NITER = ET * 8 * 2  # pass-1 iterations total

                    for hp in range(ET):
                        if hp > 0:
                            fill_to(ready_mark[hp])
                        for kt in range(8):
                            klo = 128 * kt
                            for hj in range(2):
                                s_ps = st_chunk(psS, hp, hj, kt, 0, True)
                                nc.scalar.activation(
                                    at_A[hp, kt, hj][:, 0:BCH - klo],
                                    s_ps[:, klo:],
                                    Exp, scale=SCALE,
                                    accum_out=sums2[hp, kt, hj][:, 0:1],
                                )
                                giter[0] += 1
                                fill_to((giter[0] * nfill) // NITER)
                    fill_to(nfill)

                # ---- phase B: attention pass 2 (q>=1024) + AV ----
                # The two heads of a pair interleave their kt loops and
                # SHARE one y^T psum tile: head A's padded AV writes zeros
                # to rows 64:128 (B's rows) and vice versa, so rows 0:64 =
                # head A's y^T and 64:128 = head B's. pend carries across
                # pairs so the PE pipeline never drains at a pair boundary.
                with tc.tile_pool(name="psY", bufs=1, space="PSUM") as psY:
                    pend = []  # [(hp, yps, hj, kt, atA, atB, vsp)]
                    n_av = [0] * ET

                    def emit_av(hp0, yps0, hj, kt0, atA0, atB0, vsp0):
                        klo0 = 128 * kt0
                        for qc in range(kt0 // 4, NQC):
                            lo = max(QCH * qc, klo0)
                            hi = QCH * qc + QCH
                            if qc < 2:
                                mov = atA0[:, lo - klo0:hi - klo0]
                            else:
                                mov = atB0[:, lo - BCH:hi - BCH]
                            nc.tensor.matmul(
                                yps0[:, lo:hi], vsp0, mov,
                                start=(kt0 == 0 and hj == 0),
                                stop=(kt0 == min(TT - 1, 4 * qc + 3) and hj == 1),
                                skip_group_check=True,
                            )
                        n_av[hp0] += 1
                        if n_av[hp0] == 16:
                            # all kt<8 AVs emitted: yps[:, :1024] is final
                            nc.vector.tensor_copy(y_sb[:, hp0, 0:BCH], yps0[:, 0:BCH])
                        elif n_av[hp0] == 2 * TT:
                            nc.vector.tensor_copy(y_sb[:, hp0, BCH:T], yps0[:, BCH:T])

                    for hp in range(ET):
                        yps = psY.tile([128, T], FP32, tag="y", name="yps")
                        for kt in range(TT):
                            klo = 128 * kt
                            for hj in range(2):
                                hl = 2 * hp + hj
                                atB = atp.tile([128, BCH], BF16, tag="atB", bufs=8, name="atB")
                                rcp = sm.tile([128, 1], FP32, tag="rcp", bufs=6, name="rcp")
                                if kt < 8:
                                    s_ps = st_chunk(psS, hp, hj, kt, 1, False)
                                    nc.scalar.activation(
                                        atB, s_ps, Exp, scale=SCALE,
                                        accum_out=sums2[hp, kt, hj][:, 1:2],
                                    )
                                    stot = sm.tile([128, 1], FP32, tag="stot", bufs=6, name="stot")
                                    nc.vector.reduce_sum(stot, sums2[hp, kt, hj], axis=AxX)
                                    nc.vector.reciprocal(rcp, stot)
                                else:
                                    s_ps = st_chunk(psS, hp, hj, kt, 1, True)
                                    sums = sm.tile([128, 1], FP32, tag="sums", bufs=6, name="sums")
                                    nc.scalar.activation(
                                        atB[:, klo - BCH:], s_ps[:, klo - BCH:],
                                        Exp, scale=SCALE, accum_out=sums,
                                    )
                                    nc.vector.reciprocal(rcp, sums)
                                # slot index must be continuous across the
                                # pair boundary: AV(pair p, kt15) is emitted
                                # during pair p+1's early iterations, after
                                # p+1's first vsp writes
                                vsp = vspad[hj][(hp * TT + kt) % 5]
                                nc.vector.tensor_scalar_mul(
                                    vsp[:, hj * 64:hj * 64 + 64],
                                    v_sb[:, kt, hl * 64:(hl + 1) * 64], rcp
                                )
                                pend.append(
                                    (hp, yps, hj, kt, at_A.get((hp, kt, hj)), atB, vsp)
                                )
                                if len(pend) > PEND:
                                    emit_av(*pend.pop(0))
                    for p_ in pend:
                        emit_av(*p_)

            # ---- output projection ----
            with tc.tile_pool(name="psP", bufs=4, space="PSUM") as psP:
                for tt in range(TT):
                    po1 = psP.tile([128, QCH], FP32, tag="ps", bufs=4, name="po1")
                    po2 = psP.tile([128, QCH], FP32, tag="ps", bufs=4, name="po2")
                    for et in range(ET):
                        nc.tensor.matmul(
                            po1, y_sb[:, et, tt * 128:(tt + 1) * 128], wp_sb[:, et, 0:QCH],
                            start=(et == 0), stop=(et == ET - 1),
                        )
                        nc.tensor.matmul(
                            po2[:, :C - QCH], y_sb[:, et, tt * 128:(tt + 1) * 128],
                            wp_sb[:, et, QCH:C],
                            start=(et == 0), stop=(et == ET - 1),
                        )
                    o_sb = op.tile([128, C], FP32, tag="o", bufs=3, name="o_sb")
                    nc.vector.tensor_copy(o_sb[:, 0:QCH], po1)
                    nc.scalar.copy(o_sb[:, QCH:C], po2[:, :C - QCH])
                    nc.sync.dma_start(out=out[tt * 128:(tt + 1) * 128, :], in_=o_sb)

    _split_sync_waits(nc)
    return nc


_nc_cache = {}
last_result = None


def kernel(x, w_attn, b_attn, w_proj, b_proj):
    global last_result
    if "nc" not in _nc_cache:
        _nc_cache["nc"] = _build()
    nc = _nc_cache["nc"]

    x = np.asarray(x, dtype=np.float32)
    w_attn = np.asarray(w_attn, dtype=np.float32)
    b_attn = np.asarray(b_attn, dtype=np.float32)
    w_proj = np.asarray(w_proj, dtype=np.float32)
    b_proj = np.asarray(b_proj, dtype=np.float32)

    bf16 = ml_dtypes.bfloat16
    tri = np.where(
        np.arange(128)[None, :] >= np.arange(128)[:, None], 0.0, MASKV
    ).astype(bf16)
    eye = np.eye(128, dtype=np.float32).astype(bf16)

    in_maps = []
    for core in range(NCORES):
        b = core // 2
        e0 = (core % 2) * E
        xt_host = np.ascontiguousarray(
            x[b].T.reshape(CT, 128, NQC, QCH).transpose(2, 1, 0, 3)
        ).astype(bf16)
        def _wblk(w):
            return np.ascontiguousarray(
                w.reshape(CT, 128, E).transpose(1, 0, 2)
            ).astype(bf16)
        in_maps.append({
            "xT": xt_host,
            "wq": _wblk(w_attn[:, e0:e0 + E]),
            "wk": _wblk(w_attn[:, C + e0:C + e0 + E]),
            "wv": _wblk(w_attn[:, 2 * C + e0:2 * C + e0 + E]),
            "bq": np.ascontiguousarray(b_attn[e0:e0 + E]),
            "bk": np.ascontiguousarray(b_attn[C + e0:C + e0 + E]),
            "bv": np.ascontiguousarray(b_attn[2 * C + e0:2 * C + e0 + E]),
            "wp": np.ascontiguousarray(
                w_proj[e0:e0 + E, :].reshape(ET, 128, C).transpose(1, 0, 2)
            ).astype(bf16),
            "mask": tri,
            "ident": eye,
        })

    trace = os.environ.get("ATT_TRACE", "0")
    kw = {}
    if trace != "0":
        n = min(int(trace), NCORES)
        kw = dict(trace=True, trace_cores=list(range(n)))
    res = run_bass_kernel_spmd(nc, in_maps, list(range(NCORES)), **kw)
    last_result = res

    out = np.zeros((B, T, C), dtype=np.float32)
    for core in range(NCORES):
        out[core // 2] += res.results[core]["out"]
    out += b_proj[None, None, :]
    return out
